# revision 1
# baseline (speedup 1.0000x reference)
"""Multi-head self-attention (RoPE, causal) Trainium2 Bass kernel.

Sharding: 8 cores = batch(2) x head-group(4). Each core computes QKV projection
for its 4 heads, RoPE, causal attention, and a partial output projection
(its 256 channels of w_o's contraction); partials are summed on the host.

Layout/engine choices:
  - All matmul operands in bf16 (halves DMA volume; PE throughput is
    1 cycle/row for bf16 at any width; rel-err ~4e-3 vs the 2e-2 budget).
  - "Transposed" activations: Q^T/K^T stored [d, s]; RoPE d-order arranged
    so the rotate-half partner sits 16 lanes away within each 32-partition
    quadrant, produced by one DVE stream_shuffle (no PE permutation
    matmul). V stored [s, d] augmented with a ones column so the softmax
    denominator rides as row 64 of the PV result.
  - Head-PAIR score tiles: heads 2p/2p+1 live in partition halves 0-63/64-127
    of Q^T/K^T, so their score matmuls target PE row-tiles T0/T8
    (64-row array tiling -> HW can overlap them) and write the two halves
    of one [128,1024] PSUM tile, evaluated by a single wide exp (strided
    2-range exp/mask ops on the causal diagonal).
  - Engine roles: ACT = exp + PSUM evacuation of Q/K projections;
    DVE = RoPE combine, masks, V evacuation, y-normalization, outproj
    evacuation; Pool(gpsimd) = denominator partition-broadcast + x/out DMA;
    SP(sync) = weight/table DMA. No DMA on ACT/DVE queues.
  - j-outer attention loop with filler interleaving: projection tiles and
    output-projection tiles are emitted inside the exp-paced attention
    streams so the in-order PE queue always has runway; DMAs are split
    into arrival-ordered pieces so the first QK pair starts at ~1.5us.
"""
import sys

if "/opt/trn_rl_repo" not in sys.path:
    sys.path.insert(0, "/opt/trn_rl_repo")

import numpy as np

D_MODEL = 1024
N_HEADS = 16
D_K = 64
THETA = 10000.0
BATCH, SEQ = 2, 2048
N_CORES = 8
HPC = 4           # heads per core
LOC = HPC * D_K   # 256 local channels
P = 128
IC = D_MODEL // P  # 8 contraction chunks

_nc_cache = {}


def _build_nc():
    import concourse.bass as bass
    import concourse.bacc as bacc
    import concourse.tile as tile
    import concourse.mybir as mybir
    from concourse import library_config

    F32 = mybir.dt.float32
    BF16 = mybir.dt.bfloat16
    MULT = mybir.AluOpType.mult
    ADD = mybir.AluOpType.add
    EXP = mybir.ActivationFunctionType.Exp

    nc = bacc.Bacc("TRN2", target_bir_lowering=False, debug=False)

    xt = nc.dram_tensor("xt", [D_MODEL, SEQ], BF16, kind="ExternalInput")
    wqk = nc.dram_tensor("wqk", [D_MODEL, 2 * LOC], BF16, kind="ExternalInput")
    wv = nc.dram_tensor("wv", [D_MODEL, LOC], BF16, kind="ExternalInput")
    wo = nc.dram_tensor("wo", [LOC, D_MODEL], BF16, kind="ExternalInput")
    c2 = nc.dram_tensor("c2", [P, SEQ], BF16, kind="ExternalInput")
    s2p = nc.dram_tensor("s2p", [P, SEQ], BF16, kind="ExternalInput")
    masks = nc.dram_tensor("masks", [P, 4, 512], BF16, kind="ExternalInput")
    out = nc.dram_tensor("out", [SEQ, D_MODEL], BF16, kind="ExternalOutput")

    with tile.TileContext(nc) as tc:
        with (
            tc.tile_pool(name="consts", bufs=1) as consts,
            tc.tile_pool(name="xtp", bufs=32) as xtp,
            tc.tile_pool(name="persist", bufs=1) as persist,
            tc.tile_pool(name="rtmpp", bufs=4) as rtmpp,
            tc.tile_pool(name="epool", bufs=8) as epool,
            tc.tile_pool(name="denp", bufs=2) as denp,
            tc.tile_pool(name="denbp", bufs=2) as denbp,
            tc.tile_pool(name="outsb", bufs=3) as outsb,
        ):
            # ---- constant loads: sync queue only (ACT/DVE stay DMA-free) ----
            wqk_sb = consts.tile([P, IC, 2 * LOC], BF16)
            wv_sb = consts.tile([P, IC, LOC], BF16)
            for ic in range(4):
                nc.sync.dma_start(wqk_sb[:, ic, :], wqk[P * ic:P * (ic + 1), :])
            nc.sync.dma_start(wv_sb[:], wv[:].rearrange("(ic p) o -> p ic o", p=P))
            for ic in range(4, IC):
                nc.sync.dma_start(wqk_sb[:, ic, :], wqk[P * ic:P * (ic + 1), :])
            # RoPE tables: first halves land before the first RoPE needs them,
            # second halves ride later in the queue
            c2_sb = consts.tile([P, SEQ], BF16)
            s2p_sb = consts.tile([P, SEQ], BF16)
            nc.sync.dma_start(c2_sb[:, 0:1024], c2[:, 0:1024])
            nc.sync.dma_start(s2p_sb[:, 0:1024], s2p[:, 0:1024])
            # RoPE partner swap: lanes i <-> i+16 within each 32-partition
            # quadrant (weight rows pre-permuted so partners are 16 apart)
            SWAP_MASK = [(i + 16) % 32 for i in range(32)]
            masks_sb = consts.tile([P, 4, 512], BF16)
            nc.sync.dma_start(masks_sb[:], masks[:])
            nc.sync.dma_start(c2_sb[:, 1024:SEQ], c2[:, 1024:SEQ])
            nc.sync.dma_start(s2p_sb[:, 1024:SEQ], s2p[:, 1024:SEQ])
            wo_sb = consts.tile([P, 2, D_MODEL], BF16)
            nc.sync.dma_start(wo_sb[:], wo[:].rearrange("(it p) o -> p it o", p=P))
            ones_sb = consts.tile([P, 16 * HPC], BF16)
            nc.vector.memset(ones_sb[:], 1.0)
            zeros_sb = consts.tile([P, 384], BF16)
            nc.vector.memset(zeros_sb[:], 0.0)
            # warm the ACT table for Exp while the initial DMAs stream, so
            # the first RoPE copy / first exp don't pay the table load
            warm_sb = consts.tile([1, 1], F32)
            nc.scalar.activation(warm_sb[:], ones_sb[0:1, 0:1], EXP)
            # warm the PE array (HAM clock gate / p-state ramp) with a
            # read-free dummy accumulation while the first weight/x DMAs
            # stream: the real projection matmuls then start at full clock
            warm_in = consts.tile([P, P], BF16)
            nc.vector.memset(warm_in[:], 0.0)

            # ---- persistent activations ----
            qt_sb = persist.tile([P, 2, SEQ], BF16)   # [64*(h%2)+d, h//2, s]
            kt_sb = persist.tile([P, 2, SEQ], BF16)
            vb_sb = persist.tile([P, 16, HPC * 65], BF16)  # [s%128, s//128, 65h+d]
            yt_sb = persist.tile([P, 2, SEQ], BF16)

            # ones column of V augmentation (softmax denominator row)
            nc.vector.tensor_copy(
                vb_sb[:].rearrange("p s (h e) -> p s h e", e=65)[:, :, :, 64],
                ones_sb[:].rearrange("p (s h) -> p s h", h=HPC),
            )

            ppA_cm = tc.tile_pool(name="ppA", bufs=2, space="PSUM")
            ppA = ppA_cm.__enter__()
            stps_cm = tc.tile_pool(name="stps", bufs=2, space="PSUM")
            stps = stps_cm.__enter__()
            pvps_cm = tc.tile_pool(name="pvps", bufs=2, space="PSUM")
            pvps = pvps_cm.__enter__()

            def load_xt(half, xts_out):
                # 512-col pieces, ncl-major: the ncl=0 pieces land first so
                # the first QK pair (and attention j=0) starts ~3us earlier
                for ncl in range(2):
                    for ic in range(IC):
                        xt_t = xtp.tile([P, 512], BF16,
                                        name=f"xt_{half}_{ic}_{ncl}", tag="xt")
                        nc.gpsimd.dma_start(
                            xt_t[:], xt[P * ic:P * (ic + 1),
                                        1024 * half + 512 * ncl:
                                        1024 * half + 512 * (ncl + 1)]
                        )
                        xts_out[(ic, ncl)] = xt_t

            def _rope(half, ot, ncl, ps):
                s0 = 1024 * half + 512 * ncl
                dst_tile = qt_sb if ot < 2 else kt_sb
                dst = dst_tile[:, ot % 2, s0:s0 + 512]
                nc.scalar.copy(dst, ps[:])
                sw = rtmpp.tile([P, 512], BF16,
                                name=f"sw_{half}_{ot}_{ncl}", tag="sw")
                nc.vector.stream_shuffle(sw[:], dst, SWAP_MASK)
                tmp = rtmpp.tile([P, 512], BF16,
                                 name=f"rt_{half}_{ot}_{ncl}", tag="rt")
                nc.vector.tensor_tensor(tmp[:], sw[:],
                                        s2p_sb[:, s0:s0 + 512], MULT)
                nc.vector.tensor_tensor(dst, dst,
                                        c2_sb[:, s0:s0 + 512], MULT)
                nc.vector.tensor_tensor(dst, dst, tmp[:], ADD)

            def qk_pair(half, ots, ncl, xts):
                # two Q/K projection tiles, ic-major (so each arriving xt
                # chunk feeds two matmuls during the initial DMA ramp), then
                # RoPE each into transposed layout [o, s]
                pss = [ppA.tile([P, 512], F32, name=f"pp_{half}_{ot}_{ncl}",
                                tag="pp") for ot in ots]
                for ic in range(IC):
                    for ps, ot in zip(pss, ots):
                        nc.tensor.matmul(
                            ps[:],
                            lhsT=wqk_sb[:, ic, P * ot:P * (ot + 1)],
                            rhs=xts[(ic, ncl)][:],
                            start=(ic == 0), stop=(ic == IC - 1),
                        )
                for ps, ot in zip(pss, ots):
                    _rope(half, ot, ncl, ps)

            def qk_tile(half, ot, ncl, xts):
                # one Q/K projection tile -> transposed layout [o, s] + RoPE
                ps = ppA.tile([P, 512], F32,
                              name=f"pp_{half}_{ot}_{ncl}", tag="pp")
                for ic in range(IC):
                    nc.tensor.matmul(
                        ps[:],
                        lhsT=wqk_sb[:, ic, P * ot:P * (ot + 1)],
                        rhs=xts[(ic, ncl)][:],
                        start=(ic == 0), stop=(ic == IC - 1),
                    )
                _rope(half, ot, ncl, ps)

            def v_tile(half, stl, xts):
                # V projection s-tile -> layout B [s, d] into augmented vb
                st = 8 * half + stl
                ps = ppA.tile([P, 512], F32, name=f"ppv_{half}_{stl}",
                              tag="pp")
                ncl, sof = stl // 4, P * (stl % 4)
                for ic in range(IC):
                    nc.tensor.matmul(
                        ps[:, 0:LOC],
                        lhsT=xts[(ic, ncl)][:, sof:sof + P],
                        rhs=wv_sb[:, ic, :],
                        start=(ic == 0), stop=(ic == IC - 1),
                    )
                # half-0 tiles evacuate on ACT (idle during the proj-0 ramp);
                # half-1 tiles on DVE (ACT is exp-paced inside j=2)
                evac = nc.scalar.copy if half == 0 else nc.vector.tensor_copy
                evac(
                    vb_sb[:, st].rearrange("p (h e) -> p h e", e=65)[:, :, 0:64],
                    ps[:, 0:LOC].rearrange("p (h e) -> p h e", e=64),
                )

            pv_tiles = {}
            _pending = []

            def _emit_st_exp(hp, i, j):
                """Score matmuls for head pair (2hp, 2hp+1): the two heads'
                STs are emitted back-to-back on PE row-tiles T0/T8 (64-row
                array tiling -> HW can overlap), each into its own 1-bank
                PSUM tile with its own exp for a 4-deep ST->exp pipeline."""
                t = hp
                diag = (i // 4 == j)
                z = P * (i % 4) if diag else 0  # fully-masked prefix width
                stp = stps.tile([P, 1024], F32, name=f"st_{hp}_{i}_{j}", tag="st")
                for hh in range(2):
                    r0 = 64 * hh
                    c0 = 512 * hh
                    nc.tensor.matmul(
                        stp[:, c0 + z:c0 + 512],
                        lhsT=kt_sb[r0:r0 + 64, t, P * i:P * (i + 1)],
                        rhs=qt_sb[r0:r0 + 64, t, 512 * j + z:512 * j + 512],
                        start=True, stop=True,
                    )
                e_t = epool.tile([P, 1024], BF16, name=f"e_{hp}_{i}_{j}", tag="e")
                e2 = e_t[:].rearrange("p (h c) -> p h c", c=512)
                s2 = stp[:].rearrange("p (h c) -> p h c", c=512)
                if z == 0:
                    nc.scalar.activation(e_t[:], stp[:], EXP, scale=0.125)
                else:
                    # both head-halves in one strided instruction
                    nc.scalar.activation(e2[:, :, z:512], s2[:, :, z:512],
                                         EXP, scale=0.125)
                if diag:
                    tm = i % 4
                    mb, _ = bass.broadcast_tensor_aps(
                        masks_sb[:, tm, z:z + P].rearrange("p (o c) -> p o c", o=1),
                        e2[:, :, z:z + P])
                    nc.vector.tensor_tensor(
                        e2[:, :, z:z + P], e2[:, :, z:z + P], mb, MULT)
                return e_t

            def _emit_pv(hp, i, j, e_t):
                t = hp
                for hh in range(2):
                    h = 2 * hp + hh
                    r0 = 64 * hh
                    c0 = 512 * hh
                    pv = pv_tiles[(h, j)]
                    # diag blocks contribute only to columns >= 128*(i%4);
                    # the masked prefix is skipped (i=0 start covers the
                    # full range, so has_written is set everywhere)
                    zz = P * (i % 4) if (i // 4 == j) else 0
                    nc.tensor.matmul(
                        pv[0:65, zz:512],
                        lhsT=vb_sb[:, i, 65 * h:65 * h + 65],
                        rhs=e_t[:, c0 + zz:c0 + 512],
                        start=(i == 0), stop=(i == 4 * j + 3),
                    )
                    if i == 4 * j + 3:
                        dn = denp.tile([1, 512], F32, name=f"dn_{h}_{j}",
                                       tag="dn")
                        nc.vector.reciprocal(dn[:], pv[64:65, :])
                        db = denbp.tile([64, 512], F32, name=f"db_{h}_{j}",
                                        tag="db")
                        nc.gpsimd.partition_broadcast(db[:], dn[:])
                        nc.vector.tensor_tensor(
                            yt_sb[r0:r0 + 64, t, 512 * j:512 * j + 512],
                            pv[0:64, :], db[:], MULT,
                        )

            def attn_chunk(hp, i, j):
                """Software-pipelined by two groups: PV trails the ST+exp
                stream so the in-order PE queue is never head-blocked
                waiting for an exp."""
                e_t = _emit_st_exp(hp, i, j)
                _pending.append((hp, i, j, e_t))
                if len(_pending) > 2:
                    _emit_pv(*_pending.pop(0))

            def attn_flush():
                while _pending:
                    _emit_pv(*_pending.pop(0))

            def attn_j(j, fillers=(), every=1):
                """Attention for q-chunk j, interleaving `fillers` (thunks of
                independent PE-heavy work) every `every` groups so the PE
                stream has runway while exps pace the attention chain.
                No flush at chunk boundaries: the ST->exp->PV pipeline runs
                straight through (PV accumulators are keyed by (head, j);
                outproj fillers wait on the ydivs via data deps)."""
                fillers = list(fillers)
                g = 0
                for hp in range(2):
                    for hh in range(2):
                        h = 2 * hp + hh
                        pv_tiles[(h, j)] = pvps.tile([P, 512], F32,
                                                     name=f"pv_{h}_{j}",
                                                     tag="pv")
                    for i in range(4 * j + 4):
                        attn_chunk(hp, i, j)
                        g += 1
                        if fillers and g % every == 0:
                            fillers.pop(0)()
                attn_flush()
                while fillers:
                    fillers.pop(0)()

            def outproj_tile(st):
                ob = outsb.tile([P, D_MODEL], BF16, name=f"ob_{st}", tag="ob")
                for oc in range(2):
                    ps = ppA.tile([P, 512], F32, name=f"ppb_{st}_{oc}",
                                  tag="pp")
                    for it in range(2):
                        nc.tensor.matmul(
                            ps[:],
                            lhsT=yt_sb[:, it, P * st:P * (st + 1)],
                            rhs=wo_sb[:, it, 512 * oc:512 * (oc + 1)],
                            start=(it == 0), stop=(it == 1),
                        )
                    # the last chunk's tiles evacuate on ACT (exps are done
                    # by then); mid-stream tiles stay on DVE
                    if st >= 12:
                        nc.scalar.copy(ob[:, 512 * oc:512 * (oc + 1)], ps[:])
                    else:
                        nc.vector.tensor_copy(ob[:, 512 * oc:512 * (oc + 1)],
                                              ps[:])
                dma_eng = (nc.sync, nc.gpsimd)[st % 2]
                dma_eng.dma_start(out[P * st:P * (st + 1), :], ob[:])

            # ---- emission ----
            # Phase A: enough of proj-half-0 to start j=0 (all ncl=0 QK tiles
            # + first 4 V s-tiles), the rest rides inside j=0/j=1 as fillers.
            xts0, xts1 = {}, {}
            load_xt(0, xts0)
            nc.gpsimd.load_library(library_config.attn)
            wpp = ppA.tile([P, 512], F32, name="pe_warm", tag="pp")
            for k in range(12):
                nc.tensor.matmul(wpp[:, 0:P], lhsT=warm_in[:], rhs=warm_in[:],
                                 start=(k == 0), stop=(k == 11))
            # (q,k) tiles for head-pair 0 first so j=0/hp=0 can start while
            # the rest of proj-half-0 rides inside j=0 as fillers
            qk_pair(0, (0, 2), 0, xts0)
            for stl in range(6):
                v_tile(0, stl, xts0)
            load_xt(1, xts1)          # gpsimd queue: runs during attention A
            fillA = [lambda: qk_pair(0, (1, 3), 0, xts0),
                     lambda: v_tile(0, 6, xts0),
                     lambda: qk_pair(0, (0, 1), 1, xts0),
                     lambda: v_tile(0, 7, xts0),
                     lambda: qk_pair(0, (2, 3), 1, xts0)]
            attn_j(0, fillA, every=1)
            fillB = [lambda st=st: outproj_tile(st) for st in range(0, 4)]
            # pre-RoPE the (q,k) tiles j=2/hp=0 needs inside j=1 so the
            # phase boundary has no projection-latency stall
            fillB += [lambda: qk_pair(1, (0, 2), 0, xts1)]
            attn_j(1, fillB, every=3)
            # Phase B: j=2 with the rest of proj-half-1 + outproj(j=1) fillers
            fillC = [lambda: qk_pair(1, (1, 3), 0, xts1),
                     lambda: qk_pair(1, (0, 1), 1, xts1),
                     lambda: qk_pair(1, (2, 3), 1, xts1)]
            fillC += [lambda stl=stl: v_tile(1, stl, xts1) for stl in range(8)]
            fillC += [lambda st=st: outproj_tile(st) for st in range(4, 8)]
            attn_j(2, fillC, every=1)
            fillD = [lambda st=st: outproj_tile(st) for st in range(8, 12)]
            attn_j(3, fillD, every=7)
            attn_flush()
            for st in range(12, 16):
                outproj_tile(st)

            pvps_cm.__exit__(None, None, None)
            stps_cm.__exit__(None, None, None)
            ppA_cm.__exit__(None, None, None)

    nc.compile()
    return nc


def get_nc(phases=None):
    if "nc" not in _nc_cache:
        _nc_cache["nc"] = _build_nc()
    return _nc_cache["nc"]


def make_in_maps(x, w_qkv, w_o, token_positions):
    """Host-side sharding: per-core input dict list (cores 0..7)."""
    import ml_dtypes
    BF = ml_dtypes.bfloat16

    x = np.asarray(x, dtype=np.float32)
    w_qkv = np.asarray(w_qkv, dtype=np.float32)
    w_o = np.asarray(w_o, dtype=np.float32)
    pos = np.asarray(token_positions).astype(np.float32)

    # RoPE tables (replicated). Row layout per 64-row head block: two
    # 32-lane quadrants [f 0..15 | f 16..31], each quadrant holding the
    # even components in lanes 0-15 and the odd components in lanes 16-31,
    # so the rotate-half partner is a fixed 16-lane swap within each
    # quadrant (DVE stream_shuffle).
    inv = THETA ** (-np.arange(32, dtype=np.float32) / 32.0)
    ang = pos[:, None] * inv[None, :]          # [SEQ, 32]
    cos_t = np.cos(ang).T.astype(np.float32)   # [32, SEQ] rows = f
    sin_t = np.sin(ang).T.astype(np.float32)
    # per-64-row block: rows r: quad = r//32, lane = r%32,
    # f = 16*quad + lane%16, sign = -1 for lanes 0-15 (even slots) else +1
    fidx = np.array([16 * (r // 32) + (r % 32) % 16 for r in range(64)])
    sgn = np.array([-1.0 if (r % 32) < 16 else 1.0 for r in range(64)],
                   dtype=np.float32)
    c64 = cos_t[fidx]                       # [64, SEQ]
    s64 = sin_t[fidx] * sgn[:, None]        # [64, SEQ]
    c2 = np.tile(c64, (2, 1)).astype(BF)    # [128, SEQ]
    s2p = np.tile(s64, (2, 1)).astype(BF)

    # causal diag masks: mask[p, t, f] = 1 if (p <= f - 128 t)
    pp = np.arange(P)[:, None]
    ff = np.arange(512)[None, :]
    masks = np.stack([(pp <= ff - P * tt).astype(BF) for tt in range(4)],
                     axis=1)  # [128, 4, 512]

    # per-head row permutation matching the table layout above:
    # quadrant q in 0..1, lane l: component index = 2*(16q + l%16) + (l>=16)
    perm = np.array([2 * (16 * (l // 32) + (l % 32) % 16) + (1 if (l % 32) >= 16 else 0)
                     for l in range(64)])

    in_maps = []
    for c in range(N_CORES):
        b, hg = c // 4, c % 4
        rows = np.concatenate([hg * LOC + hh * D_K + perm for hh in range(HPC)])
        wq_p = w_qkv[rows, :]                       # [256, 1024] permuted q rows
        wk_p = w_qkv[D_MODEL + rows, :]             # [256, 1024] permuted k rows
        wv_c = w_qkv[2 * D_MODEL + hg * LOC: 2 * D_MODEL + (hg + 1) * LOC, :]
        in_maps.append({
            "xt": np.ascontiguousarray(x[b].T).astype(BF),
            "wqk": np.ascontiguousarray(
                np.concatenate([wq_p, wk_p], axis=0).T).astype(BF),
            "wv": np.ascontiguousarray(wv_c.T).astype(BF),
            "wo": np.ascontiguousarray(
                w_o[:, hg * LOC:(hg + 1) * LOC].T).astype(BF),
            "c2": c2,
            "s2p": s2p,
            "masks": np.ascontiguousarray(masks),
        })
    return in_maps


def combine_outputs(results):
    """results: list of 8 dicts with 'out' [SEQ, D_MODEL] bf16 partials."""
    out = np.zeros((BATCH, SEQ, D_MODEL), dtype=np.float32)
    for c, r in enumerate(results):
        out[c // 4] += np.asarray(r["out"]).astype(np.float32)
    return out


def kernel(x, w_qkv, w_o, token_positions):
    from concourse.bass_utils import run_bass_kernel_spmd

    nc = get_nc()
    in_maps = make_in_maps(x, w_qkv, w_o, token_positions)
    res = run_bass_kernel_spmd(nc, in_maps, list(range(N_CORES)))
    return combine_outputs(res.results)

